# revision 12
# baseline (speedup 1.0000x reference)
"""Trainium2 Bass kernel for an attention layer whose math collapses.

The module computes softmax over a size-1 axis, so the attention weights
are exactly 1.0 and the output is context[b, 0, d] = sum_t a[b, t, d].
The MLP branch (W1, b1, W2, b2) and s_prev never affect the output.

Strategy: pure data parallel over the batch axis. Each of the 8 cores
reduces its [16, 512, 512] shard over the time axis on the tensor
engine: each 128x128 data tile is the stationary operand, a constant
ones vector is the moving operand, so per-column time-sums accumulate
in a single PSUM bank. All HBM reads are fully contiguous 1 MiB DMAs.
Memory-bound: ~16 MiB HBM read per core (~47 us at ~358 GB/s).

Raw Bass (not Tile): the HW allows very few sync-waits per instruction,
which fights Tile's auto-generated waits; with two manual counting
semaphores (DMA loads, PE matmuls) every wait is a standalone
single-condition instruction and the Tile tail barriers are avoided.
"""

from contextlib import ExitStack

import numpy as np

B, TX, D = 128, 512, 512
N_CORES = 8
NB = B // N_CORES  # 16 batches per core
P = 128            # SBUF partitions
NCHUNK = TX // P   # 4 time-chunks of 128
NDBLK = D // P     # 4 d-blocks of 128

_CACHE: dict = {}


def _build_bass():
    import concourse.bass as bass
    import concourse.mybir as mybir

    f32 = mybir.dt.float32
    nc = bass.Bass("TRN2")
    a = nc.dram_tensor("a", [NB, TX, D], f32, kind="ExternalInput")
    # out[p, b*NDBLK + j] = sum_t a[b, t, j*128 + p]; host re-layouts.
    out = nc.dram_tensor("out", [P, NB * NDBLK], f32, kind="ExternalOutput")

    ones = nc.const_aps.aps[(f32, 1.0)]  # preamble-initialized [128, 1]
    a_t = a.rearrange("b (c p) d -> b p c d", p=P)  # [NB, P, NCHUNK, D]

    n_mm = NB * NDBLK * NCHUNK  # 256

    with (
        nc.sbuf_tensor([P, NB * NCHUNK * D], f32) as abuf,
        nc.sbuf_tensor([P, NB * NDBLK], f32) as ost,
        nc.psum_tensor([P, NB * NDBLK], f32) as ps,
        nc.semaphore("pe_sem") as pe_sem,
        nc.semaphore("cp_sem") as cp_sem,
        nc.semaphore("st_sem") as st_sem,
        ExitStack() as _sems,
    ):
        # One completion semaphore per load DMA: completions of concurrent
        # DMAs are unordered, so a shared counting sem would be racy.
        dma_sems = [
            _sems.enter_context(nc.semaphore(f"dma_sem{b}")) for b in range(NB)
        ]
        block = _sems.enter_context(nc.Block(no_gpsimd_drain=True))
        abuf_t = abuf[:].rearrange("p (b c d) -> p b c d", b=NB, c=NCHUNK)

        @block.sync
        def _(sync):
            for b in range(NB):
                # 1 MiB load: [128 partitions x 4 chunks x 512], each
                # (p, c) row is a contiguous 2 KiB DRAM read.
                sync.dma_start(out=abuf_t[:, b], in_=a_t[b]).then_inc(dma_sems[b], 16)
            sync.wait_ge(cp_sem, 1)
            sync.dma_start(out=out[:], in_=ost[:]).then_inc(st_sem, 16)
            sync.wait_ge(st_sem, 16)

        @block.vector
        def _(vector):
            # DMA cannot read PSUM; bounce through SBUF on the DVE.
            vector.wait_ge(pe_sem, n_mm)
            vector.tensor_copy(ost[:], ps[:]).then_inc(cp_sem, 1)

        @block.tensor
        def _(tensor):
            for b in range(NB):
                tensor.wait_ge(dma_sems[b], 16)
                for j in range(NDBLK):
                    col = b * NDBLK + j
                    for c in range(NCHUNK):
                        tensor.matmul(
                            ps[:, col : col + 1],
                            lhsT=abuf_t[:, b, c, j * P : (j + 1) * P],
                            rhs=ones[:, 0:1],
                            start=(c == 0),
                            stop=(c == NCHUNK - 1),
                        ).then_inc(pe_sem, 1)

    return nc


def _get_bass():
    if "nc" not in _CACHE:
        _CACHE["nc"] = _build_bass()
    return _CACHE["nc"]


def _unshard(out_core: np.ndarray) -> np.ndarray:
    # [P, NB*NDBLK] -> [NB, D]: out[b, j*128 + p] = out_core[p, b*NDBLK + j]
    return out_core.reshape(P, NB, NDBLK).transpose(1, 2, 0).reshape(NB, D)


def run_spmd(a, **spmd_kwargs):
    """Run the SPMD kernel on all 8 cores; returns (full_output, BassKernelResults)."""
    from concourse.bass_utils import run_bass_kernel_spmd

    nc = _get_bass()
    a = np.ascontiguousarray(np.asarray(a), dtype=np.float32)
    assert a.shape == (B, TX, D), a.shape
    in_maps = [{"a": a[k * NB : (k + 1) * NB]} for k in range(N_CORES)]
    res = run_bass_kernel_spmd(nc, in_maps, list(range(N_CORES)), **spmd_kwargs)
    out = np.concatenate(
        [_unshard(res.results[k]["out"]) for k in range(N_CORES)], axis=0
    )
    return out.reshape(B, 1, D).astype(np.float32), res


def kernel(a, s_prev=None, W1=None, b1=None, W2=None, b2=None, **_unused):
    out, _ = run_spmd(a)
    return out


# revision 15
# speedup vs baseline: 1.4373x; 1.4373x over previous
"""Trainium2 Bass kernel for an attention layer whose math collapses.

The module computes softmax over a size-1 axis, so the attention weights
are exactly 1.0 and the output is context[b, 0, d] = sum_t a[b, t, d].
The MLP branch (W1, b1, W2, b2) and s_prev never affect the output.

Strategy: pure data parallel over the batch axis; each of the 8 cores
reduces its [16, 512, 512] shard over the time axis. Memory-bound:
~16 MiB HBM read per core (~47 us roofline at ~358 GB/s).

Kernel shape (per core):
  - The 16 MiB shard is loaded as 8 slabs of 2 MiB, each DMA'd as
    [128 partitions x 16 KiB contiguous] (large descriptors, all 16
    SDMA engines engaged). Loads alternate between the two HWDGE rings
    (SP and Activation sequencers) so per-DMA fixed costs overlap.
  - Each slab holds 2 full batches (64 partitions each). A host-
    provided block-indicator matrix ind[128, 2] (1.0 where p//64 == m)
    is the stationary matmul operand: 8 accumulating matmuls per slab
    reduce it to psum[2, 512] = per-batch time-sums.
  - DVE bounces each psum bank to SBUF; one 32 KiB store writes
    out[2, 8*512]; the host re-layouts to [16, 512].

Raw Bass (not Tile): the HW allows very few sync-waits per instruction,
which fights Tile's auto-generated waits; with per-DMA completion
semaphores every wait is a standalone single-condition instruction and
Tile's tail barriers are avoided.
"""

from contextlib import ExitStack

import numpy as np

B, TX, D = 128, 512, 512
N_CORES = 8
NB = B // N_CORES   # 16 batches per core
P = 128             # SBUF partitions
NSLAB = 8           # 2 MiB DMA slabs per core
BPS = NB // NSLAB   # batches per slab = 2
FPP = NB * TX * D // (NSLAB * P)  # f32 per partition per slab = 4096
RPS = FPP // D      # rhs matmuls per slab = 8
PPB = P // BPS      # partitions per batch within a slab = 64

_CACHE: dict = {}


def _build_bass():
    import concourse.bass as bass
    import concourse.mybir as mybir

    f32 = mybir.dt.float32
    nc = bass.Bass("TRN2")
    a = nc.dram_tensor("a", [NB, TX, D], f32, kind="ExternalInput")
    ind = nc.dram_tensor("ind", [P, BPS], f32, kind="ExternalInput")
    # out[q, g*D + d] = sum_t a[g*BPS + q, t, d]; host re-layouts.
    out = nc.dram_tensor("out", [BPS, NSLAB * D], f32, kind="ExternalOutput")

    a_sl = a.rearrange("b t d -> (b t d)").rearrange("(g p f) -> g p f", g=NSLAB, p=P)
    n_mm = NSLAB * RPS  # 64

    with ExitStack() as ctx:
        abuf = ctx.enter_context(nc.sbuf_tensor([P, NSLAB * FPP], f32))
        indb = ctx.enter_context(nc.sbuf_tensor([P, BPS], f32))
        ost = ctx.enter_context(nc.sbuf_tensor([BPS, NSLAB * D], f32))
        psb = [
            ctx.enter_context(nc.psum_tensor(f"ps{g}", [BPS, D], f32))
            for g in range(NSLAB)
        ]
        # One completion semaphore per DMA: concurrent DMA completions
        # are unordered, so a shared counting sem would be racy.
        ld_sems = [
            ctx.enter_context(nc.semaphore(f"ld_sem{g}")) for g in range(NSLAB)
        ]
        ind_sem = ctx.enter_context(nc.semaphore("ind_sem"))
        pe_sem = ctx.enter_context(nc.semaphore("pe_sem"))
        cp_sem = ctx.enter_context(nc.semaphore("cp_sem"))
        st_sem = ctx.enter_context(nc.semaphore("st_sem"))
        block = ctx.enter_context(nc.Block(no_gpsimd_drain=True))

        abuf_t = abuf[:].rearrange("p (g f) -> p g f", g=NSLAB)

        @block.sync
        def _(sync):
            sync.dma_start(out=indb[:], in_=ind[:]).then_inc(ind_sem, 16)
            for g in range(0, NSLAB, 2):
                sync.dma_start(out=abuf_t[:, g], in_=a_sl[g]).then_inc(ld_sems[g], 16)
            sync.wait_ge(cp_sem, NSLAB)
            sync.dma_start(out=out[:], in_=ost[:]).then_inc(st_sem, 16)
            sync.wait_ge(st_sem, 16)

        @block.scalar
        def _(scalar):
            # Second HWDGE ring (Activation sequencer) for the odd slabs.
            for g in range(1, NSLAB, 2):
                scalar.dma_start(out=abuf_t[:, g], in_=a_sl[g]).then_inc(
                    ld_sems[g], 16
                )

        @block.tensor
        def _(tensor):
            tensor.wait_ge(ind_sem, 16)
            for g in range(NSLAB):
                tensor.wait_ge(ld_sems[g], 16)
                for r in range(RPS):
                    tensor.matmul(
                        psb[g][:],
                        lhsT=indb[:],
                        rhs=abuf_t[:, g, r * D : (r + 1) * D],
                        start=(r == 0),
                        stop=(r == RPS - 1),
                    ).then_inc(pe_sem, 1)

        @block.vector
        def _(vector):
            # DMA cannot read PSUM; bounce each bank through SBUF.
            for g in range(NSLAB):
                vector.wait_ge(pe_sem, RPS * (g + 1))
                vector.tensor_copy(
                    ost[:, g * D : (g + 1) * D], psb[g][:]
                ).then_inc(cp_sem, 1)

    return nc


def _get_bass():
    if "nc" not in _CACHE:
        _CACHE["nc"] = _build_bass()
    return _CACHE["nc"]


def _indicator() -> np.ndarray:
    ind = np.zeros((P, BPS), dtype=np.float32)
    for m in range(BPS):
        ind[m * PPB : (m + 1) * PPB, m] = 1.0
    return ind


def _unshard(out_core: np.ndarray) -> np.ndarray:
    # [BPS, NSLAB*D] -> [NB, D]: batch g*BPS + q is at out_core[q, g*D:(g+1)*D]
    return out_core.reshape(BPS, NSLAB, D).transpose(1, 0, 2).reshape(NB, D)


def run_spmd(a, **spmd_kwargs):
    """Run the SPMD kernel on all 8 cores; returns (full_output, BassKernelResults)."""
    from concourse.bass_utils import run_bass_kernel_spmd

    nc = _get_bass()
    a = np.ascontiguousarray(np.asarray(a), dtype=np.float32)
    assert a.shape == (B, TX, D), a.shape
    ind = _indicator()
    in_maps = [
        {"a": a[k * NB : (k + 1) * NB], "ind": ind} for k in range(N_CORES)
    ]
    res = run_bass_kernel_spmd(nc, in_maps, list(range(N_CORES)), **spmd_kwargs)
    out = np.concatenate(
        [_unshard(res.results[k]["out"]) for k in range(N_CORES)], axis=0
    )
    return out.reshape(B, 1, D).astype(np.float32), res


def kernel(a, s_prev=None, W1=None, b1=None, W2=None, b2=None, **_unused):
    out, _ = run_spmd(a)
    return out


# revision 19
# speedup vs baseline: 1.4410x; 1.0026x over previous
"""Trainium2 Bass kernel for an attention layer whose math collapses.

The module computes softmax over a size-1 axis, so the attention weights
are exactly 1.0 and the output is context[b, 0, d] = sum_t a[b, t, d].
The MLP branch (W1, b1, W2, b2) and s_prev never affect the output.

Strategy: pure data parallel over the batch axis; each of the 8 cores
reduces its [16, 512, 512] shard over the time axis. Memory-bound:
~16 MiB HBM read per core (~41 us measured at ~415 GB/s aggregate).

Kernel shape (per core):
  - The 16 MiB shard is loaded as 8 slabs of 2 MiB, each DMA'd as
    [128 partitions x 16 KiB contiguous] (large descriptors, all 16
    SDMA engines engaged). Loads alternate between the two HWDGE rings
    (SP and Activation sequencers) so per-DMA fixed costs overlap.
  - Each slab holds 2 full batches (64 partitions each), 8 time-rows
    of 512 per partition. The DVE folds each slab in place with 3
    contiguous halving adds (4096 -> 512 f32 per partition); fp32
    matmuls are ~1.2 us/512 cols (HI/LO split), so keeping the bulk
    reduction off the PE is a ~4x win there.
  - One fp32 matmul per slab with a host-provided block-indicator
    ind[128, 2] (1.0 where p//64 == m) as the stationary operand
    finishes the cross-partition reduction into psum[2, 512].
  - ACT bounces each psum bank to SBUF; one 32 KiB store writes
    out[2, 8*512]; the host re-layouts to [16, 512].

Raw Bass (not Tile): the HW allows very few sync-waits per instruction,
which fights Tile's auto-generated waits; with per-DMA completion
semaphores every wait is a standalone single-condition instruction and
Tile's tail barriers are avoided.
"""

from contextlib import ExitStack

import numpy as np

B, TX, D = 128, 512, 512
N_CORES = 8
NB = B // N_CORES   # 16 batches per core
P = 128             # SBUF partitions
NSLAB = 8           # 2 MiB DMA slabs per core
BPS = NB // NSLAB   # batches per slab = 2
FPP = NB * TX * D // (NSLAB * P)  # f32 per partition per slab = 4096
PPB = P // BPS      # partitions per batch within a slab = 64

_CACHE: dict = {}


def _build_bass():
    import concourse.bass as bass
    import concourse.mybir as mybir

    f32 = mybir.dt.float32
    add = mybir.AluOpType.add
    nc = bass.Bass("TRN2")
    a = nc.dram_tensor("a", [NB, TX, D], f32, kind="ExternalInput")
    ind = nc.dram_tensor("ind", [P, BPS], f32, kind="ExternalInput")
    # out[q, g*D + d] = sum_t a[g*BPS + q, t, d]; host re-layouts.
    out = nc.dram_tensor("out", [BPS, NSLAB * D], f32, kind="ExternalOutput")

    a_sl = a.rearrange("b t d -> (b t d)").rearrange(
        "(g p f) -> g p f", g=NSLAB, p=P
    )

    with ExitStack() as ctx:
        abuf = ctx.enter_context(nc.sbuf_tensor([P, NSLAB * FPP], f32))
        red = [
            ctx.enter_context(nc.sbuf_tensor(f"red{g}", [P, D], f32))
            for g in range(NSLAB)
        ]
        indb = ctx.enter_context(nc.sbuf_tensor([P, BPS], f32))
        ost = ctx.enter_context(nc.sbuf_tensor([BPS, NSLAB * D], f32))
        psb = [
            ctx.enter_context(nc.psum_tensor(f"ps{g}", [BPS, D], f32))
            for g in range(NSLAB)
        ]
        # One completion semaphore per DMA: concurrent DMA completions
        # are unordered, so a shared counting sem would be racy.
        ld_sems = [
            ctx.enter_context(nc.semaphore(f"ld_sem{g}")) for g in range(NSLAB)
        ]
        red_sems = [
            ctx.enter_context(nc.semaphore(f"red_sem{g}")) for g in range(NSLAB)
        ]
        ind_sem = ctx.enter_context(nc.semaphore("ind_sem"))
        pe_sem = ctx.enter_context(nc.semaphore("pe_sem"))
        cp_sem = ctx.enter_context(nc.semaphore("cp_sem"))
        st_sem = ctx.enter_context(nc.semaphore("st_sem"))
        block = ctx.enter_context(nc.Block(no_gpsimd_drain=True))

        abuf_t = abuf[:].rearrange("p (g f) -> p g f", g=NSLAB)

        @block.sync
        def _(sync):
            sync.dma_start(out=indb[:], in_=ind[:]).then_inc(ind_sem, 16)
            for g in range(0, NSLAB, 2):
                sync.dma_start(out=abuf_t[:, g], in_=a_sl[g]).then_inc(ld_sems[g], 16)
            sync.wait_ge(cp_sem, NSLAB)
            sync.dma_start(out=out[:], in_=ost[:]).then_inc(st_sem, 16)
            sync.wait_ge(st_sem, 16)

        @block.scalar
        def _(scalar):
            # Second HWDGE ring (Activation sequencer) for the odd slabs.
            for g in range(1, NSLAB, 2):
                scalar.dma_start(out=abuf_t[:, g], in_=a_sl[g]).then_inc(
                    ld_sems[g], 16
                )
            # ACT also bounces finished psum banks to SBUF (DMA cannot
            # read PSUM; DVE is busy folding slabs).
            for g in range(NSLAB):
                scalar.wait_ge(pe_sem, g + 1)
                scalar.copy(ost[:, g * D : (g + 1) * D], psb[g][:]).then_inc(
                    cp_sem, 1
                )

        @block.vector
        def _(vector):
            # Fold each slab 4096 -> 512 f32/partition: one X-axis reduce
            # over the 8 time-rows (innermost, stride 512).
            for g in range(NSLAB):
                vector.wait_ge(ld_sems[g], 16)
                src = abuf_t[:, g].rearrange("p (r d) -> p d r", r=FPP // D)
                vector.tensor_reduce(
                    red[g][:], src, mybir.AxisListType.X, add
                ).then_inc(red_sems[g], 1)

        @block.tensor
        def _(tensor):
            tensor.wait_ge(ind_sem, 16)
            for g in range(NSLAB):
                tensor.wait_ge(red_sems[g], 1)
                tensor.matmul(
                    psb[g][:],
                    lhsT=indb[:],
                    rhs=red[g][:],
                    start=True,
                    stop=True,
                ).then_inc(pe_sem, 1)

    return nc


def _get_bass():
    if "nc" not in _CACHE:
        _CACHE["nc"] = _build_bass()
    return _CACHE["nc"]


def _indicator() -> np.ndarray:
    ind = np.zeros((P, BPS), dtype=np.float32)
    for m in range(BPS):
        ind[m * PPB : (m + 1) * PPB, m] = 1.0
    return ind


def _unshard(out_core: np.ndarray) -> np.ndarray:
    # [BPS, NSLAB*D] -> [NB, D]: batch g*BPS + q is at out_core[q, g*D:(g+1)*D]
    return out_core.reshape(BPS, NSLAB, D).transpose(1, 0, 2).reshape(NB, D)


def run_spmd(a, **spmd_kwargs):
    """Run the SPMD kernel on all 8 cores; returns (full_output, BassKernelResults)."""
    from concourse.bass_utils import run_bass_kernel_spmd

    nc = _get_bass()
    a = np.ascontiguousarray(np.asarray(a), dtype=np.float32)
    assert a.shape == (B, TX, D), a.shape
    ind = _indicator()
    in_maps = [
        {"a": a[k * NB : (k + 1) * NB], "ind": ind} for k in range(N_CORES)
    ]
    res = run_bass_kernel_spmd(nc, in_maps, list(range(N_CORES)), **spmd_kwargs)
    out = np.concatenate(
        [_unshard(res.results[k]["out"]) for k in range(N_CORES)], axis=0
    )
    return out.reshape(B, 1, D).astype(np.float32), res


def kernel(a, s_prev=None, W1=None, b1=None, W2=None, b2=None, **_unused):
    out, _ = run_spmd(a)
    return out


# revision 20
# speedup vs baseline: 1.7227x; 1.1955x over previous
"""Trainium2 Bass kernel for an attention layer whose math collapses.

The module computes softmax over a size-1 axis, so the attention weights
are exactly 1.0 and the output is context[b, 0, d] = sum_t a[b, t, d].
The MLP branch (W1, b1, W2, b2) and s_prev never affect the output.

Strategy: pure data parallel over the batch axis; each of the 8 cores
reduces its [16, 512, 512] shard over the time axis. Memory-bound:
~16 MiB HBM read per core (~40 us window at ~420 GB/s aggregate over
both HWDGE rings).

Kernel shape (per core):
  - The 16 MiB shard is loaded as 8 slabs of 2 MiB, each DMA'd as
    [128 partitions x 16 KiB contiguous] (large descriptors, all 16
    SDMA engines engaged). Even slabs go on the SP HWDGE ring, odd
    slabs on the Activation ring, so per-DMA fixed costs overlap.
  - Each slab holds 2 full batches (64 partitions each), 8 time-rows
    of 512 per partition. Measured engine rates: fp32 PE matmul is
    ~1.2us per 512 cols (HI/LO split, 75us for all data - too slow),
    DVE tensor_reduce is 1x-mode with a stride penalty (7us/slab).
    Fastest is 3 contiguous in-place halving adds per slab
    (4096->2048->1024->512): ~4.3us on DVE, ~2x that on GPSIMD.
    Early-arriving slabs go to GPSIMD, the rest to DVE, so both
    finish inside the DMA window and the last slab folds on the
    faster DVE. Chained same-engine adds need an intra-slab semaphore
    handshake (deep pipelines have no RAW interlock).
  - One fp32 matmul per slab with a host-provided block-indicator
    ind[128, 2] (1.0 where p//64 == m) as the stationary operand
    finishes the cross-partition reduction into psum[2, 512].
  - ACT bounces each psum bank to SBUF; per-slab 4 KiB stores overlap
    all but the last store's latency. Host re-layouts to [16, 512].

Raw Bass (not Tile): the HW allows very few sync-waits per instruction,
which fights Tile's auto-generated waits; with per-DMA completion
semaphores every wait is a standalone single-condition instruction and
Tile's tail barriers are avoided.
"""

from contextlib import ExitStack

import numpy as np

B, TX, D = 128, 512, 512
N_CORES = 8
NB = B // N_CORES   # 16 batches per core
P = 128             # SBUF partitions
NSLAB = 8           # 2 MiB DMA slabs per core
BPS = NB // NSLAB   # batches per slab = 2
FPP = NB * TX * D // (NSLAB * P)  # f32 per partition per slab = 4096
PPB = P // BPS      # partitions per batch within a slab = 64

# Slabs folded on GPSIMD (early arrivals; ~2x slower than DVE) vs DVE.
POOL_SLABS = (0, 1, 2)

_CACHE: dict = {}


def _build_bass():
    import concourse.bass as bass
    import concourse.mybir as mybir

    f32 = mybir.dt.float32
    add = mybir.AluOpType.add
    nc = bass.Bass("TRN2")
    a = nc.dram_tensor("a", [NB, TX, D], f32, kind="ExternalInput")
    ind = nc.dram_tensor("ind", [P, BPS], f32, kind="ExternalInput")
    # out[q, g*D + d] = sum_t a[g*BPS + q, t, d]; host re-layouts.
    out = nc.dram_tensor("out", [BPS, NSLAB * D], f32, kind="ExternalOutput")

    a_sl = a.rearrange("b t d -> (b t d)").rearrange(
        "(g p f) -> g p f", g=NSLAB, p=P
    )

    with ExitStack() as ctx:
        abuf = ctx.enter_context(nc.sbuf_tensor([P, NSLAB * FPP], f32))
        indb = ctx.enter_context(nc.sbuf_tensor([P, BPS], f32))
        ost = ctx.enter_context(nc.sbuf_tensor([BPS, NSLAB * D], f32))
        psb = [
            ctx.enter_context(nc.psum_tensor(f"ps{g}", [BPS, D], f32))
            for g in range(NSLAB)
        ]
        # One completion semaphore per DMA: concurrent DMA completions
        # are unordered, so a shared counting sem would be racy.
        ld_sems = [
            ctx.enter_context(nc.semaphore(f"ld_sem{g}")) for g in range(NSLAB)
        ]
        fold_sems = [
            ctx.enter_context(nc.semaphore(f"fold_sem{g}")) for g in range(NSLAB)
        ]
        red_sems = [
            ctx.enter_context(nc.semaphore(f"red_sem{g}")) for g in range(NSLAB)
        ]
        st_sems = [
            ctx.enter_context(nc.semaphore(f"st_sem{g}")) for g in range(NSLAB)
        ]
        ind_sem = ctx.enter_context(nc.semaphore("ind_sem"))
        pe_sem = ctx.enter_context(nc.semaphore("pe_sem"))
        cp_sem = ctx.enter_context(nc.semaphore("cp_sem"))
        block = ctx.enter_context(nc.Block(no_gpsimd_drain=True))

        abuf_t = abuf[:].rearrange("p (g f) -> p g f", g=NSLAB)

        def fold_slab(eng, g):
            """3 in-place contiguous halving adds: 4096 -> 512 f32/partition.
            Same-engine RAW needs an explicit sem handshake per step."""
            eng.wait_ge(ld_sems[g], 16)
            sl = abuf_t[:, g]
            steps = (FPP // 2, FPP // 4, FPP // 8)
            for k, h in enumerate(steps):
                i = eng.tensor_tensor(sl[:, 0:h], sl[:, 0:h], sl[:, h : 2 * h], add)
                if k < len(steps) - 1:
                    i.then_inc(fold_sems[g], 1)
                    eng.wait_ge(fold_sems[g], k + 1)
                else:
                    i.then_inc(red_sems[g], 1)

        @block.sync
        def _(sync):
            for g in range(0, NSLAB, 2):
                sync.dma_start(out=abuf_t[:, g], in_=a_sl[g]).then_inc(ld_sems[g], 16)
            # Per-slab 4 KiB stores: all but the last store's latency
            # overlaps with remaining compute.
            for g in range(NSLAB):
                sync.wait_ge(cp_sem, g + 1)
                sync.dma_start(
                    out=out[:, g * D : (g + 1) * D], in_=ost[:, g * D : (g + 1) * D]
                ).then_inc(st_sems[g], 16)
            for g in range(NSLAB):
                sync.wait_ge(st_sems[g], 16)

        @block.scalar
        def _(scalar):
            # Second HWDGE ring (Activation sequencer) for the odd slabs.
            for g in range(1, NSLAB, 2):
                scalar.dma_start(out=abuf_t[:, g], in_=a_sl[g]).then_inc(
                    ld_sems[g], 16
                )
            # ACT also bounces finished psum banks to SBUF (DMA cannot
            # read PSUM; DVE/GPSIMD are busy folding slabs).
            for g in range(NSLAB):
                scalar.wait_ge(pe_sem, g + 1)
                scalar.copy(ost[:, g * D : (g + 1) * D], psb[g][:]).then_inc(
                    cp_sem, 1
                )

        @block.gpsimd
        def _(gpsimd):
            # Idle Q7: load the tiny indicator off the critical rings,
            # then help fold the early slabs.
            gpsimd.dma_start(out=indb[:], in_=ind[:]).then_inc(ind_sem, 16)
            for g in POOL_SLABS:
                fold_slab(gpsimd, g)

        @block.vector
        def _(vector):
            for g in range(NSLAB):
                if g not in POOL_SLABS:
                    fold_slab(vector, g)

        @block.tensor
        def _(tensor):
            tensor.wait_ge(ind_sem, 16)
            for g in range(NSLAB):
                tensor.wait_ge(red_sems[g], 1)
                tensor.matmul(
                    psb[g][:],
                    lhsT=indb[:],
                    rhs=abuf_t[:, g, 0:D],
                    start=True,
                    stop=True,
                ).then_inc(pe_sem, 1)

    return nc


def _get_bass():
    if "nc" not in _CACHE:
        _CACHE["nc"] = _build_bass()
    return _CACHE["nc"]


def _indicator() -> np.ndarray:
    ind = np.zeros((P, BPS), dtype=np.float32)
    for m in range(BPS):
        ind[m * PPB : (m + 1) * PPB, m] = 1.0
    return ind


def _unshard(out_core: np.ndarray) -> np.ndarray:
    # [BPS, NSLAB*D] -> [NB, D]: batch g*BPS + q is at out_core[q, g*D:(g+1)*D]
    return out_core.reshape(BPS, NSLAB, D).transpose(1, 0, 2).reshape(NB, D)


def run_spmd(a, **spmd_kwargs):
    """Run the SPMD kernel on all 8 cores; returns (full_output, BassKernelResults)."""
    from concourse.bass_utils import run_bass_kernel_spmd

    nc = _get_bass()
    a = np.ascontiguousarray(np.asarray(a), dtype=np.float32)
    assert a.shape == (B, TX, D), a.shape
    ind = _indicator()
    in_maps = [
        {"a": a[k * NB : (k + 1) * NB], "ind": ind} for k in range(N_CORES)
    ]
    res = run_bass_kernel_spmd(nc, in_maps, list(range(N_CORES)), **spmd_kwargs)
    out = np.concatenate(
        [_unshard(res.results[k]["out"]) for k in range(N_CORES)], axis=0
    )
    return out.reshape(B, 1, D).astype(np.float32), res


def kernel(a, s_prev=None, W1=None, b1=None, W2=None, b2=None, **_unused):
    out, _ = run_spmd(a)
    return out


# revision 23
# speedup vs baseline: 2.1523x; 1.2494x over previous
"""Trainium2 Bass kernel for an attention layer whose math collapses.

The module computes softmax over a size-1 axis, so the attention weights
are exactly 1.0 and the output is context[b, 0, d] = sum_t a[b, t, d].
The MLP branch (W1, b1, W2, b2) and s_prev never affect the output.

Strategy: pure data parallel over the batch axis; each of the 8 cores
reduces its [16, 512, 512] shard over the time axis. Memory-bound:
~16 MiB HBM read per core (~38 us window at ~440 GB/s aggregate over
both HWDGE rings).

Kernel shape (per core):
  - The 16 MiB shard is loaded as 16 slabs of 1 MiB (one batch each),
    DMA'd as [128 partitions x 8 KiB contiguous] (large descriptors,
    all 16 SDMA engines engaged). Even slabs go on the SP HWDGE ring,
    odd slabs on the Activation ring, so per-DMA fixed costs overlap
    and slabs arrive every ~2.4 us.
  - Each slab holds one batch: 4 time-rows of 512 per partition.
    Measured engine rates: fp32 PE matmul is ~1.2 us per 512 cols
    (HI/LO split; streaming everything through the PE costs 75 us),
    DVE tensor_reduce is 1x-mode with a stride penalty. Fastest is 2
    contiguous in-place halving adds per slab (2048 -> 1024 -> 512,
    ~1.9 us on DVE, ~2x that on GPSIMD). Early slabs fold on GPSIMD,
    the rest on the faster DVE, so both keep up with arrivals and the
    last slab folds fast. Chained same-engine adds need a semaphore
    handshake (deep pipelines have no RAW interlock).
  - One fp32 matmul per slab against the preamble's constant ones
    [128, 1] vector reduces across partitions into a psum row. Eight
    psum banks hold 2 slab results each at partition offsets {0, 32}
    (PE output base partition is limited to {0, 32, 64}).
  - ACT bounces each psum row to SBUF; per-slab 2 KiB stores overlap
    all but the last store's latency.

Raw Bass (not Tile): the HW allows very few sync-waits per instruction,
which fights Tile's auto-generated waits; with per-DMA completion
semaphores every wait is a standalone single-condition instruction and
Tile's tail barriers are avoided.
"""

from contextlib import ExitStack

import numpy as np

B, TX, D = 128, 512, 512
N_CORES = 8
NB = B // N_CORES   # 16 batches per core
P = 128             # SBUF partitions
NSLAB = 16          # 1 MiB DMA slabs per core (= one batch per slab)
FPP = NB * TX * D // (NSLAB * P)  # f32 per partition per slab = 2048

# Slabs folded on GPSIMD (early arrivals; ~2x slower than DVE) vs DVE.
POOL_SLABS = (0, 1, 2, 3)

_CACHE: dict = {}


def _build_bass():
    import concourse.bass as bass
    import concourse.mybir as mybir

    f32 = mybir.dt.float32
    add = mybir.AluOpType.add
    nc = bass.Bass("TRN2")
    a = nc.dram_tensor("a", [NB, TX, D], f32, kind="ExternalInput")
    out = nc.dram_tensor("out", [NB, D], f32, kind="ExternalOutput")

    ones = nc.const_aps.aps[(f32, 1.0)]  # preamble-initialized [128, 1]
    a_sl = a.rearrange("b t d -> (b t d)").rearrange(
        "(g p f) -> g p f", g=NSLAB, p=P
    )

    with ExitStack() as ctx:
        abuf = ctx.enter_context(nc.sbuf_tensor([P, NSLAB * FPP], f32))
        ost = ctx.enter_context(nc.sbuf_tensor([1, NB * D], f32))
        psb = [
            ctx.enter_context(nc.psum_tensor(f"ps{i}", [64, D], f32))
            for i in range(8)
        ]
        # One completion semaphore per DMA: concurrent DMA completions
        # are unordered, so a shared counting sem would be racy.
        ld_sems = [
            ctx.enter_context(nc.semaphore(f"ld_sem{g}")) for g in range(NSLAB)
        ]
        fold_sems = [
            ctx.enter_context(nc.semaphore(f"fold_sem{g}")) for g in range(NSLAB)
        ]
        red_sems = [
            ctx.enter_context(nc.semaphore(f"red_sem{g}")) for g in range(NSLAB)
        ]
        st_sems = [
            ctx.enter_context(nc.semaphore(f"st_sem{g}")) for g in range(NSLAB)
        ]
        pe_sem = ctx.enter_context(nc.semaphore("pe_sem"))
        cp_sem = ctx.enter_context(nc.semaphore("cp_sem"))
        block = ctx.enter_context(nc.Block(no_gpsimd_drain=True))

        abuf_t = abuf[:].rearrange("p (g f) -> p g f", g=NSLAB)

        def fold_slab(eng, g):
            """2 in-place contiguous halving adds: 2048 -> 512 f32/partition.
            Same-engine RAW needs an explicit sem handshake per step."""
            eng.wait_ge(ld_sems[g], 16)
            sl = abuf_t[:, g]
            h = FPP // 2
            eng.tensor_tensor(sl[:, 0:h], sl[:, 0:h], sl[:, h : 2 * h], add).then_inc(
                fold_sems[g], 1
            )
            eng.wait_ge(fold_sems[g], 1)
            h = FPP // 4
            eng.tensor_tensor(sl[:, 0:h], sl[:, 0:h], sl[:, h : 2 * h], add).then_inc(
                red_sems[g], 1
            )

        @block.sync
        def _(sync):
            for g in range(0, NSLAB, 2):
                sync.dma_start(out=abuf_t[:, g], in_=a_sl[g]).then_inc(ld_sems[g], 16)
            # Per-slab 2 KiB stores: all but the last store's latency
            # overlaps with remaining compute.
            for g in range(NSLAB):
                sync.wait_ge(cp_sem, g + 1)
                sync.dma_start(
                    out=out[g : g + 1, :], in_=ost[0:1, g * D : (g + 1) * D]
                ).then_inc(st_sems[g], 16)
            for g in range(NSLAB):
                sync.wait_ge(st_sems[g], 16)

        @block.scalar
        def _(scalar):
            # Second HWDGE ring (Activation sequencer) for the odd slabs.
            for g in range(1, NSLAB, 2):
                scalar.dma_start(out=abuf_t[:, g], in_=a_sl[g]).then_inc(
                    ld_sems[g], 16
                )
            # ACT also bounces finished psum rows to SBUF (DMA cannot
            # read PSUM; DVE/GPSIMD are busy folding slabs).
            for g in range(NSLAB):
                off = 32 * (g % 2)
                scalar.wait_ge(pe_sem, g + 1)
                scalar.copy(
                    ost[:, g * D : (g + 1) * D], psb[g // 2][off : off + 1, :]
                ).then_inc(cp_sem, 1)

        @block.gpsimd
        def _(gpsimd):
            for g in POOL_SLABS:
                fold_slab(gpsimd, g)

        @block.vector
        def _(vector):
            for g in range(NSLAB):
                if g not in POOL_SLABS:
                    fold_slab(vector, g)

        @block.tensor
        def _(tensor):
            for g in range(NSLAB):
                off = 32 * (g % 2)
                tensor.wait_ge(red_sems[g], 1)
                tensor.matmul(
                    psb[g // 2][off : off + 1, :],
                    lhsT=ones[:, 0:1],
                    rhs=abuf_t[:, g, 0:D],
                    start=True,
                    stop=True,
                ).then_inc(pe_sem, 1)

    return nc


def _get_bass():
    if "nc" not in _CACHE:
        _CACHE["nc"] = _build_bass()
    return _CACHE["nc"]


def run_spmd(a, **spmd_kwargs):
    """Run the SPMD kernel on all 8 cores; returns (full_output, BassKernelResults)."""
    from concourse.bass_utils import run_bass_kernel_spmd

    nc = _get_bass()
    a = np.ascontiguousarray(np.asarray(a), dtype=np.float32)
    assert a.shape == (B, TX, D), a.shape
    in_maps = [{"a": a[k * NB : (k + 1) * NB]} for k in range(N_CORES)]
    res = run_bass_kernel_spmd(nc, in_maps, list(range(N_CORES)), **spmd_kwargs)
    out = np.concatenate([res.results[k]["out"] for k in range(N_CORES)], axis=0)
    return out.reshape(B, 1, D).astype(np.float32), res


def kernel(a, s_prev=None, W1=None, b1=None, W2=None, b2=None, **_unused):
    out, _ = run_spmd(a)
    return out
